# revision 42
# baseline (speedup 1.0000x reference)
"""Trainium2 Bass kernel for nn_ChunkLayer (ragged sequence compaction).

Per batch row: boundary-masked rows of hidden_states are compacted to the
front (stable order), padded out to M = max(per-row boundary count) with the
first non-boundary rows (matching the reference's stable-argsort semantics).

Sharding: pure data parallel — row b -> NeuronCore b (B == 8 == n_cores).
On-device work per core: dma_gather (SWDGE, mlp ucode library) pulls the
selected 4 KiB rows HBM->SBUF in chunks through a ring of SBUF buffers,
while two HWDGE engines (sync + scalar) alternate streaming completed
chunks SBUF->HBM, so gathers and writeouts fully overlap. Host work is
limited to computing the data-dependent output width M (as the reference
does eagerly), the per-row gather order (int16 index lists fed to the
device), and trimming the 128-row-padded device output back to M.
"""

import numpy as np

B, L, D = 8, 8192, 1024

# Pipeline schedule: per-chunk gathered-row counts (each a multiple of 128).
# Built at runtime from M; see _chunk_schedule.
WARMUP = 256          # small first chunk -> earlier first writeout
CHUNK = 512           # steady-state chunk rows
N_WRITERS = 2         # 1 = sync only, 2 = sync+scalar (HWDGE engines)
NBUF = 6              # gather/writeout ring depth
TRACE = False


N_FINE = 13           # number of 256-row chunks in the fine back half


SCHED_OVERRIDE = None  # test hook: explicit row-count schedule


def _chunk_schedule(m):
    """Row counts per chunk: small warmup, coarse 512-row front half, then a
    fine 256-row back half so the writers drain in lockstep with the gather
    stream (writer time per chunk < gather time -> no end-of-pipeline lag)."""
    if SCHED_OVERRIDE is not None:
        assert sum(SCHED_OVERRIDE) == -(-m // 128) * 128
        return list(SCHED_OVERRIDE)
    rows = -(-m // 128) * 128
    sched = []
    if WARMUP and rows > WARMUP:
        sched.append(WARMUP)
        rows -= WARMUP
    n_fine = min(rows // 256, N_FINE)
    coarse = rows - 256 * n_fine
    while coarse >= CHUNK:
        sched.append(CHUNK)
        coarse -= CHUNK
    if coarse:
        sched.append(coarse)
    sched.extend([256] * n_fine)
    return sched


def _build_program(sched, m=None):
    from contextlib import ExitStack

    from concourse import mybir
    from concourse.bacc import Bacc

    ni = sum(sched)                  # total rows gathered (>= M, <= L)
    max_rows = max(sched) // 128     # sbuf buffer depth (rows per partition)
    n_chunks = len(sched)
    nbuf = min(NBUF, n_chunks)
    col0 = np.concatenate([[0], np.cumsum(sched)])  # chunk starts, in rows
    if m is None:
        m = ni
    # rows the chunk actually has to gather (only the final chunk is partial)
    valid = [min(c, m - int(col0[i])) for i, c in enumerate(sched)]
    assert all(v >= 1 for v in valid)

    nc = Bacc()
    hs_d = nc.declare_dram_parameter("hs", [L, D], mybir.dt.float32, isOutput=False)
    idx_d = nc.declare_dram_parameter(
        "idx", [128, ni // 16], mybir.dt.int16, isOutput=False
    )
    out_d = nc.declare_dram_parameter(
        "out", [ni // 128, 128, D], mybir.dt.float32, isOutput=True
    )

    with ExitStack() as ctx:
        idx_sb = ctx.enter_context(
            nc.sbuf_tensor("idx_sb", [128, ni // 16], mybir.dt.int16)
        )
        bufs = [
            ctx.enter_context(
                nc.sbuf_tensor(f"buf{b}", [128, max_rows, D], mybir.dt.float32)
            )
            for b in range(nbuf)
        ]
        block = ctx.enter_context(nc.Block())
        s_idx = ctx.enter_context(nc.semaphore("s_idx"))
        s_g = [ctx.enter_context(nc.semaphore(f"s_g{b}")) for b in range(nbuf)]
        # per-(writer, buffer) semaphores: each is updated by exactly one
        # engine's DMAs (race-detector requirement)
        s_w = [
            [ctx.enter_context(nc.semaphore(f"s_w{p}_{b}")) for b in range(nbuf)]
            for p in range(N_WRITERS)
        ]

        def n_prior(i):
            """how many earlier chunks used chunk i's buffer"""
            return i // nbuf

        # Every chunk's writeout is split across both writers; for odd block
        # counts the extra block alternates between writers (i parity) so
        # cumulative load stays balanced. Each piece is
        # (phase, r_lo_block, n_blocks, rem_partitions).
        def pieces_of(i):
            v = valid[i]
            full, rem = v // 128, v % 128
            out = []
            if N_WRITERS == 1:
                out.append((0, 0, full, rem))
                return out
            hi = full // 2
            lo = full - hi
            if lo:
                out.append((0, 0, lo, 0))
            if hi or rem:
                out.append((1, lo, hi, rem))
            return out

        def w_cum(p, b, upto):
            """16-incs on s_w[p][b] from writeouts of chunks < upto"""
            n = 0
            for j in range(upto):
                if j % nbuf != b:
                    continue
                for ph, _, nb, rem in pieces_of(j):
                    if ph == p:
                        n += (1 if nb else 0) + (1 if rem else 0)
            return n

        @block.gpsimd
        def _(gpsimd):
            gpsimd.wait_ge(s_idx, 16)
            for i, c in enumerate(sched):
                b = i % nbuf
                v = valid[i]
                if i >= nbuf:
                    for p in range(N_WRITERS):
                        want = w_cum(p, b, i)
                        if want:
                            gpsimd.wait_ge(s_w[p][b], 16 * want)
                gpsimd.dma_gather(
                    bufs[b][:, : -(-v // 128), :],
                    hs_d[:, :],
                    idx_sb[:, col0[i] // 16 : col0[i] // 16 + -(-v // 16)],
                    num_idxs=v,
                    num_idxs_reg=v,
                    elem_size=D,
                ).then_inc(s_g[b], 16)

        def writer(eng, phase):
            for i, c in enumerate(sched):
                mine = [pc for pc in pieces_of(i) if pc[0] == phase]
                if not mine:
                    continue
                b = i % nbuf
                eng.wait_ge(s_g[b], 16 * (n_prior(i) + 1))
                r0 = col0[i] // 128
                for _, lo, nb, rem in mine:
                    if nb:
                        eng.dma_start(
                            out=out_d[r0 + lo : r0 + lo + nb, :, :].transpose(
                                [1, 0, 2]
                            ),
                            in_=bufs[b][:, lo : lo + nb, :],
                        ).then_inc(s_w[phase][b], 16)
                    if rem:
                        fb = lo + nb
                        eng.dma_start(
                            out=out_d[r0 + fb : r0 + fb + 1, :rem, :].transpose(
                                [1, 0, 2]
                            ),
                            in_=bufs[b][:rem, fb : fb + 1, :],
                        ).then_inc(s_w[phase][b], 16)
            # drain: wait for this engine's own last writeout on each buffer
            for b in range(nbuf):
                want = w_cum(phase, b, n_chunks)
                if want:
                    eng.wait_ge(s_w[phase][b], 16 * want)

        @block.sync
        def _(sync):
            sync.dma_start(out=idx_sb[:, :], in_=idx_d[:, :]).then_inc(s_idx, 16)
            writer(sync, 0)

        if N_WRITERS >= 2 and any(
            pc[0] == 1 for i in range(n_chunks) for pc in pieces_of(i)
        ):

            @block.scalar
            def _(scalar):
                writer(scalar, 1)

    nc.finalize()
    return nc


def _make_idx_map(bm_row, ni):
    order = np.concatenate([np.flatnonzero(bm_row), np.flatnonzero(~bm_row)])
    idx_full = order[:ni].astype(np.int16)
    # wrapped-16 layout: index j lives at (partition j%16, col j//16),
    # replicated across the 8 Q7 cores' 16-partition blocks
    wrapped = np.ascontiguousarray(idx_full.reshape(ni // 16, 16).T)
    return np.ascontiguousarray(np.tile(wrapped, (8, 1)))


def kernel(hidden_states, boundary_mask, mask):
    from concourse.bass_utils import run_bass_kernel_spmd

    hs = np.ascontiguousarray(np.asarray(hidden_states, dtype=np.float32))
    bm = np.asarray(boundary_mask).astype(bool)
    assert hs.shape == (B, L, D), hs.shape
    assert bm.shape == (B, L), bm.shape

    num_tokens = bm.sum(axis=1).astype(np.int64)          # (B,)
    m = int(num_tokens.max())
    if m == 0:
        return (
            np.empty((B, 0, D), dtype=np.float32),
            np.empty((B, 0), dtype=bool),
        )
    sched = _chunk_schedule(m)
    ni = sum(sched)
    assert ni <= L

    nc = _build_program(sched, m)
    in_maps = [
        {"hs": hs[b], "idx": _make_idx_map(bm[b], ni)} for b in range(B)
    ]
    try:
        res = run_bass_kernel_spmd(nc, in_maps, list(range(B)), trace=TRACE)
    except Exception:
        # transient NRT/axon failures (device unrecoverable, mesh desync)
        # are retryable; the program itself is deterministic
        res = run_bass_kernel_spmd(nc, in_maps, list(range(B)), trace=TRACE)

    out = np.empty((B, m, D), dtype=np.float32)
    for b in range(B):
        full = res.results[b]["out"].reshape(ni, D)
        out[b] = full[:m]
    next_mask = np.arange(m)[None, :] < num_tokens[:, None]

    if TRACE:
        kernel.last_exec_time_ns = res.exec_time_ns
        kernel.last_results = res
    return out, next_mask


# revision 44
# speedup vs baseline: 1.0024x; 1.0024x over previous
"""Trainium2 Bass kernel for nn_ChunkLayer (ragged sequence compaction).

Per batch row: boundary-masked rows of hidden_states are compacted to the
front (stable order), padded out to M = max(per-row boundary count) with the
first non-boundary rows (matching the reference's stable-argsort semantics).

Sharding: pure data parallel — row b -> NeuronCore b (B == 8 == n_cores).
On-device work per core: dma_gather (SWDGE, mlp ucode library) pulls the
selected 4 KiB rows HBM->SBUF in chunks through a ring of SBUF buffers,
while two HWDGE engines (sync + scalar) alternate streaming completed
chunks SBUF->HBM, so gathers and writeouts fully overlap. Host work is
limited to computing the data-dependent output width M (as the reference
does eagerly), the per-row gather order (int16 index lists fed to the
device), and trimming the 128-row-padded device output back to M.
"""

import numpy as np

B, L, D = 8, 8192, 1024

# Pipeline schedule: per-chunk gathered-row counts (each a multiple of 128).
# Built at runtime from M; see _chunk_schedule.
WARMUP = 256          # small first chunk -> earlier first writeout
CHUNK = 512           # steady-state chunk rows
N_WRITERS = 2         # 1 = sync only, 2 = sync+scalar (HWDGE engines)
NBUF = 6              # gather/writeout ring depth
TRACE = False


N_FINE = 13           # number of 256-row chunks in the fine back half


SCHED_OVERRIDE = None  # test hook: explicit row-count schedule


def _chunk_schedule(m):
    """Row counts per chunk: small warmup, coarse 512-row front half, then a
    fine 256-row back half so the writers drain in lockstep with the gather
    stream (writer time per chunk < gather time -> no end-of-pipeline lag)."""
    if SCHED_OVERRIDE is not None:
        assert sum(SCHED_OVERRIDE) == -(-m // 128) * 128
        return list(SCHED_OVERRIDE)
    rows = -(-m // 128) * 128
    sched = []
    if WARMUP and rows > WARMUP:
        sched.append(WARMUP)
        rows -= WARMUP
    n_fine = min(rows // 256, N_FINE)
    coarse = rows - 256 * n_fine
    while coarse >= CHUNK:
        sched.append(CHUNK)
        coarse -= CHUNK
    if coarse:
        sched.append(coarse)
    sched.extend([256] * n_fine)
    return sched


def _build_program(sched, m=None):
    from contextlib import ExitStack

    from concourse import mybir
    from concourse.bacc import Bacc

    ni = sum(sched)                  # total rows gathered (>= M, <= L)
    max_rows = max(sched) // 128     # sbuf buffer depth (rows per partition)
    n_chunks = len(sched)
    nbuf = min(NBUF, n_chunks)
    col0 = np.concatenate([[0], np.cumsum(sched)])  # chunk starts, in rows
    if m is None:
        m = ni
    # per-chunk (output start row, allocated rows, rows actually gathered);
    # only the final output segment is partial. Issue the partial segment
    # FIRST (it doubles as pipeline warmup) so the stream ends on a clean
    # full chunk instead of an odd-sized straggler.
    segs = [
        (int(col0[i]), c, min(c, m - int(col0[i]))) for i, c in enumerate(sched)
    ]
    assert all(v >= 1 for _, _, v in segs)
    if len(segs) > 1 and segs[-1][2] < segs[-1][1]:
        segs = [segs[-1]] + segs[:-1]
    start = [s for s, _, _ in segs]
    valid = [v for _, _, v in segs]

    nc = Bacc()
    hs_d = nc.declare_dram_parameter("hs", [L, D], mybir.dt.float32, isOutput=False)
    idx_d = nc.declare_dram_parameter(
        "idx", [128, ni // 16], mybir.dt.int16, isOutput=False
    )
    out_d = nc.declare_dram_parameter(
        "out", [ni // 128, 128, D], mybir.dt.float32, isOutput=True
    )

    with ExitStack() as ctx:
        idx_sb = ctx.enter_context(
            nc.sbuf_tensor("idx_sb", [128, ni // 16], mybir.dt.int16)
        )
        bufs = [
            ctx.enter_context(
                nc.sbuf_tensor(f"buf{b}", [128, max_rows, D], mybir.dt.float32)
            )
            for b in range(nbuf)
        ]
        block = ctx.enter_context(nc.Block())
        s_idx = ctx.enter_context(nc.semaphore("s_idx"))
        s_g = [ctx.enter_context(nc.semaphore(f"s_g{b}")) for b in range(nbuf)]
        # per-(writer, buffer) semaphores: each is updated by exactly one
        # engine's DMAs (race-detector requirement)
        s_w = [
            [ctx.enter_context(nc.semaphore(f"s_w{p}_{b}")) for b in range(nbuf)]
            for p in range(N_WRITERS)
        ]

        def n_prior(i):
            """how many earlier chunks used chunk i's buffer"""
            return i // nbuf

        # Every chunk's writeout is split across both writers; for odd block
        # counts the extra block alternates between writers (i parity) so
        # cumulative load stays balanced. Each piece is
        # (phase, r_lo_block, n_blocks, rem_partitions).
        def pieces_of(i):
            v = valid[i]
            full, rem = v // 128, v % 128
            out = []
            if N_WRITERS == 1:
                out.append((0, 0, full, rem))
                return out
            hi = full // 2
            lo = full - hi
            if lo:
                out.append((0, 0, lo, 0))
            if hi or rem:
                out.append((1, lo, hi, rem))
            return out

        def w_cum(p, b, upto):
            """16-incs on s_w[p][b] from writeouts of chunks < upto"""
            n = 0
            for j in range(upto):
                if j % nbuf != b:
                    continue
                for ph, _, nb, rem in pieces_of(j):
                    if ph == p:
                        n += (1 if nb else 0) + (1 if rem else 0)
            return n

        @block.gpsimd
        def _(gpsimd):
            gpsimd.wait_ge(s_idx, 16)
            for i, (st, c, v) in enumerate(segs):
                b = i % nbuf
                if i >= nbuf:
                    for p in range(N_WRITERS):
                        want = w_cum(p, b, i)
                        if want:
                            gpsimd.wait_ge(s_w[p][b], 16 * want)
                gpsimd.dma_gather(
                    bufs[b][:, : -(-v // 128), :],
                    hs_d[:, :],
                    idx_sb[:, st // 16 : st // 16 + -(-v // 16)],
                    num_idxs=v,
                    num_idxs_reg=v,
                    elem_size=D,
                ).then_inc(s_g[b], 16)

        def writer(eng, phase):
            for i, (st, c, v) in enumerate(segs):
                mine = [pc for pc in pieces_of(i) if pc[0] == phase]
                if not mine:
                    continue
                b = i % nbuf
                eng.wait_ge(s_g[b], 16 * (n_prior(i) + 1))
                r0 = st // 128
                for _, lo, nb, rem in mine:
                    if nb:
                        eng.dma_start(
                            out=out_d[r0 + lo : r0 + lo + nb, :, :].transpose(
                                [1, 0, 2]
                            ),
                            in_=bufs[b][:, lo : lo + nb, :],
                        ).then_inc(s_w[phase][b], 16)
                    if rem:
                        fb = lo + nb
                        eng.dma_start(
                            out=out_d[r0 + fb : r0 + fb + 1, :rem, :].transpose(
                                [1, 0, 2]
                            ),
                            in_=bufs[b][:rem, fb : fb + 1, :],
                        ).then_inc(s_w[phase][b], 16)
            # drain: wait for this engine's own last writeout on each buffer
            for b in range(nbuf):
                want = w_cum(phase, b, n_chunks)
                if want:
                    eng.wait_ge(s_w[phase][b], 16 * want)

        @block.sync
        def _(sync):
            sync.dma_start(out=idx_sb[:, :], in_=idx_d[:, :]).then_inc(s_idx, 16)
            writer(sync, 0)

        if N_WRITERS >= 2 and any(
            pc[0] == 1 for i in range(n_chunks) for pc in pieces_of(i)
        ):

            @block.scalar
            def _(scalar):
                writer(scalar, 1)

    nc.finalize()
    return nc


def _make_idx_map(bm_row, ni):
    order = np.concatenate([np.flatnonzero(bm_row), np.flatnonzero(~bm_row)])
    idx_full = order[:ni].astype(np.int16)
    # wrapped-16 layout: index j lives at (partition j%16, col j//16),
    # replicated across the 8 Q7 cores' 16-partition blocks
    wrapped = np.ascontiguousarray(idx_full.reshape(ni // 16, 16).T)
    return np.ascontiguousarray(np.tile(wrapped, (8, 1)))


def kernel(hidden_states, boundary_mask, mask):
    from concourse.bass_utils import run_bass_kernel_spmd

    hs = np.ascontiguousarray(np.asarray(hidden_states, dtype=np.float32))
    bm = np.asarray(boundary_mask).astype(bool)
    assert hs.shape == (B, L, D), hs.shape
    assert bm.shape == (B, L), bm.shape

    num_tokens = bm.sum(axis=1).astype(np.int64)          # (B,)
    m = int(num_tokens.max())
    if m == 0:
        return (
            np.empty((B, 0, D), dtype=np.float32),
            np.empty((B, 0), dtype=bool),
        )
    sched = _chunk_schedule(m)
    ni = sum(sched)
    assert ni <= L

    nc = _build_program(sched, m)
    in_maps = [
        {"hs": hs[b], "idx": _make_idx_map(bm[b], ni)} for b in range(B)
    ]
    try:
        res = run_bass_kernel_spmd(nc, in_maps, list(range(B)), trace=TRACE)
    except Exception:
        # transient NRT/axon failures (device unrecoverable, mesh desync)
        # are retryable; the program itself is deterministic
        res = run_bass_kernel_spmd(nc, in_maps, list(range(B)), trace=TRACE)

    out = np.empty((B, m, D), dtype=np.float32)
    for b in range(B):
        full = res.results[b]["out"].reshape(ni, D)
        out[b] = full[:m]
    next_mask = np.arange(m)[None, :] < num_tokens[:, None]

    if TRACE:
        kernel.last_exec_time_ns = res.exec_time_ns
        kernel.last_results = res
    return out, next_mask
